# revision 5
# baseline (speedup 1.0000x reference)
"""Trainium2 Bass kernel: BiologicalAttention (mask-modulated multi-head attention).

Full computation:
    qkv = x @ W_qkv + b_qkv                         [B, N, 3, H, D]
    S   = (q @ k^T) * D**-0.5 * (0.1 + 0.9*mask)    [B, H, N, N]
    P   = softmax(S, axis=-1)
    out = (P @ v) reshaped to [B, N, C]
    y   = out @ W_out + b_out

Sharding (8 cores): core c handles batch b = c//2 and a 4-head group
g = c%2 (heads 4g..4g+3).  Each core computes a partial y for its batch
(its heads' contribution to the output projection); the host sums the
two partials per batch and adds b_out.

On-core layout (all fp32):
  - qT/kT stored transposed [4*32, N] with head h on partitions 32h..32h+31,
    so QK^T runs as 4 concurrent K=32 row-tiled matmuls (tile_position).
  - Scores are computed TRANSPOSED: T[m, n] = sum_d k[m,d] q[n,d], so the
    softmax denominator (sum over keys m = partitions) comes from a matmul:
    V is stored [m, d] with a ones-column appended, so P@[v|1] yields both
    the attention output (rows 0..31) and the softmax sums (row 32).
  - The pre-softmax mask multiply streams from PSUM through the DVE
    (tensor_mul) and exp runs on the scalar engine.
  - Normalization is deferred past P@V: O_h and sums_h are scaled by
    1/sums_h (per query n) just before the output projection.
  - Host pre-folds scale into the mask: maskT = ((0.1+0.9*mask)*D^-0.5).T,
    and pre-augments weights with bias rows (x^T gets a ones row).
"""

import numpy as np
from contextlib import ExitStack

import concourse.bass as bass
import concourse.tile as tile
import concourse.mybir as mybir
from concourse import bacc

f32 = mybir.dt.float32
Act = mybir.ActivationFunctionType

# problem shape (hardcoded per contract)
B, N, C, H = 4, 2048, 256, 8
D = 32
SCALE = D ** -0.5
HPC = 4                # heads per core
HD = HPC * D           # 128
VW = HPC * (D + 1)     # 132: per-m-tile v-store width ([v_h | 1] x 4 heads)
NCORES = 8


def build_program(n=N, debug=False):
    """Build the SPMD Bass program for one core's shard. Same program runs
    on all 8 cores with different input bindings."""
    NQ = 4                 # n (query) chunks
    CH = n // NQ           # 512 at full size
    MT = n // 128          # m-tiles (key tiles)
    NQ2 = NQ // 2          # n-chunks per half (psum_O budget: 4 banks)

    nc = bacc.Bacc("TRN2", target_bir_lowering=False, debug=debug)

    xT_d = nc.dram_tensor("xT", [C, n], f32, kind="ExternalInput")
    maskT_d = nc.dram_tensor("maskT", [n, n], f32, kind="ExternalInput")
    wq_d = nc.dram_tensor("wq", [C + 1, HD], f32, kind="ExternalInput")
    wk_d = nc.dram_tensor("wk", [C + 1, HD], f32, kind="ExternalInput")
    wv_d = nc.dram_tensor("wv", [C + 1, VW], f32, kind="ExternalInput")
    wo_d = nc.dram_tensor("wo", [HD, C], f32, kind="ExternalInput")
    y_d = nc.dram_tensor("y", [n, C], f32, kind="ExternalOutput")

    with tile.TileContext(nc) as tc, ExitStack() as ctx:
        const = ctx.enter_context(tc.tile_pool(name="const", bufs=1))
        maskp = ctx.enter_context(tc.tile_pool(name="maskp", bufs=3))
        tpool = ctx.enter_context(tc.tile_pool(name="tpool", bufs=3))
        ppool = ctx.enter_context(tc.tile_pool(name="ppool", bufs=3))
        ypool = ctx.enter_context(tc.tile_pool(name="ypool", bufs=2))
        spool = ctx.enter_context(tc.tile_pool(name="spool", bufs=4))
        rpool = ctx.enter_context(tc.tile_pool(name="rpool", bufs=2))
        psT = ctx.enter_context(tc.tile_pool(name="psT", bufs=2, space="PSUM"))
        psO = ctx.enter_context(tc.tile_pool(name="psO", bufs=4, space="PSUM"))

        # ---------------- constants / inputs ----------------
        xc0 = const.tile([128, n], f32, tag="xc0")
        xc1 = const.tile([128, n], f32, tag="xc1")
        nc.sync.dma_start(xc0[:], xT_d[0:128, :])
        nc.sync.dma_start(xc1[:], xT_d[128:256, :])
        ones_row = const.tile([1, n], f32, tag="ones_row")
        nc.vector.memset(ones_row[:], 1.0)
        zrow = const.tile([1, CH], f32, tag="zrow")
        nc.vector.memset(zrow[:], 0.0)
        zc = const.tile([1, 128], f32, tag="zc")
        nc.vector.memset(zc[:], 0.0)

        wq_sb = const.tile([128, 2 * HD], f32, tag="wq_sb")
        wk_sb = const.tile([128, 2 * HD], f32, tag="wk_sb")
        wv_sb = const.tile([128, 2 * VW], f32, tag="wv_sb")
        wqb = const.tile([1, HD], f32, tag="wqb")
        wkb = const.tile([1, HD], f32, tag="wkb")
        wvb = const.tile([1, VW], f32, tag="wvb")
        wo_sb = const.tile([128, C], f32, tag="wo_sb")
        for sb, d_, w in ((wq_sb, wq_d, HD), (wk_sb, wk_d, HD), (wv_sb, wv_d, VW)):
            nc.sync.dma_start(sb[:, 0:w], d_[0:128, :])
            nc.sync.dma_start(sb[:, w:2 * w], d_[128:256, :])
        nc.sync.dma_start(wqb[:], wq_d[256:257, :])
        nc.sync.dma_start(wkb[:], wk_d[256:257, :])
        nc.sync.dma_start(wvb[:], wv_d[256:257, :])
        nc.sync.dma_start(wo_sb[:], wo_d[:])

        qT_sb = const.tile([128, n], f32, tag="qT_sb")
        kT_sb = const.tile([128, n], f32, tag="kT_sb")
        v_store = const.tile([128, MT * VW], f32, tag="v_store")
        O_allT = const.tile([128, n], f32, tag="O_allT")

        # ---------------- phase 1: QKV projections ----------------
        # qT/kT: [head*32+d, n] = W.T @ x.T  (+ bias via ones-row rank-1 term)
        for chunk in range(NQ):
            cs = bass.ts(chunk, CH)
            for dst, w_sb, w_b in ((qT_sb, wq_sb, wqb), (kT_sb, wk_sb, wkb)):
                pq = psO.tile([128, CH], f32, tag="psO")
                nc.tensor.matmul(pq[:], lhsT=w_sb[:, 0:HD], rhs=xc0[:, cs],
                                 start=True, stop=False)
                nc.tensor.matmul(pq[:], lhsT=w_sb[:, HD:2 * HD], rhs=xc1[:, cs],
                                 start=False, stop=False)
                nc.tensor.matmul(pq[:], lhsT=w_b[0:1, :], rhs=ones_row[0:1, cs],
                                 start=False, stop=True)
                nc.scalar.copy(dst[:, cs], pq[:])
        # v: [m, (v_h | 1) x 4] per m-tile; ones column comes from the bias row
        for t in range(MT):
            ms = bass.ts(t, 128)
            pv = psO.tile([128, VW], f32, tag="psO")
            nc.tensor.matmul(pv[:], lhsT=xc0[:, ms], rhs=wv_sb[:, 0:VW],
                             start=True, stop=False)
            nc.tensor.matmul(pv[:], lhsT=xc1[:, ms], rhs=wv_sb[:, VW:2 * VW],
                             start=False, stop=False)
            nc.tensor.matmul(pv[:], lhsT=ones_row[0:1, ms], rhs=wvb[0:1, :],
                             start=False, stop=True)
            nc.scalar.copy(v_store[:, t * VW:(t + 1) * VW], pv[:])

        # ---------------- phase 2: attention main loop ----------------
        for half in range(2):
            # psum_O accumulators for this half: [nq2][pair], heads 2*pair
            # at partitions 0:33 and 2*pair+1 at 64:97 (col-tiled PV).
            po = [[psO.tile([128, CH], f32, name="po", tag="psO") for _ in range(2)]
                  for _ in range(NQ2)]
            for nq2 in range(NQ2):
                for pair in range(2):
                    nc.tensor.matmul(po[nq2][pair][:], lhsT=zc[0:1, :],
                                     rhs=zrow[0:1, :], start=True, stop=True)
            for t in range(MT):
                mask_t = maskp.tile([128, NQ2 * CH], f32, tag="mask_t")
                nc.sync.dma_start(
                    mask_t[:],
                    maskT_d[t * 128:(t + 1) * 128,
                            half * NQ2 * CH:(half + 1) * NQ2 * CH])
                for nq2 in range(NQ2):
                    nq = half * NQ2 + nq2
                    qs = bass.ts(nq, CH)
                    T_tile = tpool.tile([128, 4 * CH], f32, tag="T_tile")
                    for pair in range(2):
                        pt = psT.tile([128, 2 * CH], f32, tag="psT")
                        for hh in range(2):
                            h = 2 * pair + hh
                            nc.tensor.matmul(
                                pt[:, hh * CH:(hh + 1) * CH],
                                lhsT=kT_sb[32 * h:32 * h + 32, t * 128:(t + 1) * 128],
                                rhs=qT_sb[32 * h:32 * h + 32, qs],
                                start=True, stop=True,
                                tile_position=(32 * h, 0))
                        # fused PSUM-evict + mask multiply (mask repeated 2x)
                        mb = mask_t[:, nq2 * CH:(nq2 + 1) * CH]
                        mrep = bass.AP(tensor=mb.tensor, offset=mb.offset,
                                       ap=[mb.ap[0], [0, 2], mb.ap[-1]])
                        nc.vector.tensor_mul(
                            T_tile[:, pair * 2 * CH:(pair + 1) * 2 * CH],
                            pt[:], mrep)
                    P_tile = ppool.tile([128, 4 * CH], f32, tag="P_tile")
                    nc.scalar.activation(P_tile[:], T_tile[:], Act.Exp)
                    for pair in range(2):
                        for hh in range(2):
                            h = 2 * pair + hh
                            p0 = 64 * hh
                            vs = v_store[:, t * VW + 33 * h:t * VW + 33 * h + 33]
                            nc.tensor.matmul(
                                po[nq2][pair][p0:p0 + 33, :],
                                lhsT=vs, rhs=P_tile[:, h * CH:(h + 1) * CH],
                                start=False, stop=(t == MT - 1),
                                tile_position=(0, p0),
                                skip_group_check=True)
            # ---- epilogue for this half: normalize, project, store ----
            for nq2 in range(NQ2):
                nq = half * NQ2 + nq2
                qs = bass.ts(nq, CH)
                # sums live at psum partitions 32 (head 2*pair) / 96 (2*pair+1);
                # copy head h's sums row to partition 32h (quadrant-aligned)
                sraw = spool.tile([128, CH], f32, tag="sraw")
                for pair in range(2):
                    for hh in range(2):
                        h = 2 * pair + hh
                        nc.scalar.copy(sraw[32 * h:32 * h + 1, :],
                                       po[nq2][pair][32 + 64 * hh:33 + 64 * hh, :])
                # broadcast each head's sums row across its 32 partitions (DMA)
                r_raw = rpool.tile([128, CH], f32, tag="r_raw")
                src = sraw[:]
                pstep = src.ap[0][0]
                bc = bass.AP(tensor=src.tensor, offset=src.offset,
                             ap=[[32 * pstep, 4], [0, 32], src.ap[-1]])
                nc.sync.dma_start(r_raw[:], bc)
                r_scr = rpool.tile([128, CH], f32, tag="r_scr")
                r_all = rpool.tile([128, CH], f32, tag="r_all")
                nc.vector.reciprocal_approx_accurate(r_all[:], r_raw[:], r_scr[:])
                # evict + normalize O^T chunks
                for pair in range(2):
                    for hh in range(2):
                        h = 2 * pair + hh
                        p0 = 64 * hh
                        nc.vector.tensor_mul(O_allT[32 * h:32 * h + 32, qs],
                                             po[nq2][pair][p0:p0 + 32, :],
                                             r_all[32 * h:32 * h + 32, :])
                # output projection for this n-chunk
                py = psT.tile([128, 2 * CH], f32, tag="psT")
                for j in range(CH // 128):
                    ncol = nq * CH + j * 128
                    nc.tensor.matmul(py[:, j * C:(j + 1) * C],
                                     lhsT=O_allT[:, ncol:ncol + 128],
                                     rhs=wo_sb[:], start=True, stop=True)
                y_sb = ypool.tile([128, (CH // 128) * C], f32, tag="y_sb")
                nc.scalar.copy(y_sb[:], py[:, 0:(CH // 128) * C])
                for j in range(CH // 128):
                    nc.sync.dma_start(
                        y_d[nq * CH + j * 128:nq * CH + (j + 1) * 128, :],
                        y_sb[:, j * C:(j + 1) * C])
    nc.finalize()
    return nc


def host_prep(x, interaction_mask, W_qkv, b_qkv, W_out, b_out, n=N):
    """Build per-core input bindings (host-side sharding + layout prep)."""
    x = np.asarray(x, np.float32)
    interaction_mask = np.asarray(interaction_mask, np.float32)
    W_qkv = np.asarray(W_qkv, np.float32)
    b_qkv = np.asarray(b_qkv, np.float32)
    W_out = np.asarray(W_out, np.float32)

    maskT = np.ascontiguousarray(
        ((0.1 + 0.9 * interaction_mask) * SCALE).T).astype(np.float32)
    Wr = W_qkv.reshape(C, 3, H, D)
    br = b_qkv.reshape(3, H, D)
    Wor = W_out.reshape(H, D, C)

    in_maps = []
    for core in range(NCORES):
        b = core // 2
        g = core % 2
        hs = slice(4 * g, 4 * g + 4)
        xT = np.ascontiguousarray(x[b].T)  # [C, n]

        wq = np.concatenate([Wr[:, 0, hs, :].reshape(C, HD),
                             br[0, hs, :].reshape(1, HD)], axis=0)
        wk = np.concatenate([Wr[:, 1, hs, :].reshape(C, HD),
                             br[1, hs, :].reshape(1, HD)], axis=0)
        # v augmented with a ones column per head: weights 0, bias 1
        wv_blocks, bv_blocks = [], []
        for h in range(4 * g, 4 * g + 4):
            wv_blocks.append(np.concatenate(
                [Wr[:, 2, h, :], np.zeros((C, 1), np.float32)], axis=1))
            bv_blocks.append(np.concatenate(
                [br[2, h, :], np.ones((1,), np.float32)]))
        wv = np.concatenate(
            [np.concatenate(wv_blocks, axis=1),
             np.concatenate(bv_blocks)[None, :]], axis=0)  # [C+1, VW]
        wo = np.ascontiguousarray(Wor[hs].reshape(HD, C))

        in_maps.append({
            "xT": np.ascontiguousarray(xT),
            "maskT": maskT,
            "wq": np.ascontiguousarray(wq),
            "wk": np.ascontiguousarray(wk),
            "wv": np.ascontiguousarray(wv),
            "wo": wo,
        })
    return in_maps


_PROGRAM = {}


def get_program():
    if "nc" not in _PROGRAM:
        _PROGRAM["nc"] = build_program()
    return _PROGRAM["nc"]


def combine_outputs(results, b_out):
    """results: list of 8 per-core {name: np.ndarray}. Sums head-group
    partials per batch and adds the output bias."""
    b_out = np.asarray(b_out, np.float32)
    out = np.empty((B, N, C), np.float32)
    for b in range(B):
        out[b] = results[2 * b]["y"] + results[2 * b + 1]["y"] + b_out[None, :]
    return out


def kernel(x, interaction_mask, W_qkv, b_qkv, W_out, b_out):
    from concourse.bass_utils import run_bass_kernel_spmd

    in_maps = host_prep(x, interaction_mask, W_qkv, b_qkv, W_out, b_out)
    nc = get_program()
    res = run_bass_kernel_spmd(nc, in_maps, list(range(NCORES)))
    return combine_outputs(res.results, b_out)


# revision 18
# speedup vs baseline: 8.4849x; 8.4849x over previous
"""Trainium2 Bass kernel: BiologicalAttention (mask-modulated multi-head attention).

Full computation:
    qkv = x @ W_qkv + b_qkv                         [B, N, 3, H, D]
    S   = (q @ k^T) * D**-0.5 * (0.1 + 0.9*mask)    [B, H, N, N]
    P   = softmax(S, axis=-1)
    out = (P @ v) reshaped to [B, N, C]
    y   = out @ W_out + b_out

Sharding (8 cores): core c handles batch b = c//2 and a 4-head group
g = c%2 (heads 4g..4g+3).  Each core computes a partial y for its batch
(its heads' contribution to the output projection); the host sums the
two partials per batch and adds b_out.

On-core layout (all data fp32; matmul operands in the hot loop are
materialized as float32r — same bytes, rounded — which streams through
the PE at 1 cycle/column instead of fp32's 4):
  - qT/kT stored transposed [4*32, N] with head h on partitions 32h..32h+31,
    so QK^T runs as 4 concurrent K=32 row-tiled matmuls (tile_position).
  - Scores are computed TRANSPOSED: T[m, n] = sum_d k[m,d] q[n,d], so the
    softmax denominator (sum over keys m = partitions) comes from a matmul:
    V is stored [m, d] with a ones-column appended, so P@[v|1] yields both
    the attention output (rows 0..31) and the softmax sums (row 32).
  - The pre-softmax mask multiply streams from PSUM through the DVE
    (tensor_mul, fused eviction) and exp runs on the scalar engine.
  - Normalization is deferred past P@V: O_h is scaled by 1/sums_h (per
    query n) while evicting PSUM, just before the output projection.
  - The n (query) axis is processed in 4 passes of 512; each pass gives
    every head its own PSUM accumulator bank, and each m-tile's mask chunk
    is streamed from HBM exactly once.
  - Host pre-folds scale into the mask: maskT = ((0.1+0.9*mask)*D^-0.5).T,
    and pre-augments weights with bias rows (x^T gets a ones row).
"""

import numpy as np
from contextlib import ExitStack

import concourse.bass as bass
import concourse.tile as tile
import concourse.mybir as mybir
from concourse import bacc

f32 = mybir.dt.float32
f32r = mybir.dt.float32r
bf16 = mybir.dt.bfloat16
Act = mybir.ActivationFunctionType

# problem shape (hardcoded per contract)
B, N, C, H = 4, 2048, 256, 8
D = 32
SCALE = D ** -0.5
HPC = 4                # heads per core
HD = HPC * D           # 128
VW = HPC * (D + 1)     # 132: per-m-tile v-store width ([v_h | 1] x 4 heads)
NCORES = 8


def build_program(n=N, debug=False, reps=1, gp_frac=0):
    """Build the SPMD Bass program for one core's shard. Same program runs
    on all 8 cores with different input bindings.

    reps: repeat the whole computation (timing aid: device time scales with
    reps while per-call dispatch overhead stays fixed).
    gp_frac: offload gp_frac/8 of the mask-multiply units to
    ACT-evict + GPSIMD-multiply (the DVE pays a pipe-drain tax ~2x its
    nominal throughput, so spreading the elementwise work pays off).
    """
    NQ = 4                 # n (query) passes
    CH = n // NQ           # 512 at full size
    MT = n // 128          # m-tiles (key tiles)
    TE = 2                 # m-tiles sharing one Exp op
    assert MT % TE == 0

    nc = bacc.Bacc("TRN2", target_bir_lowering=False, debug=debug)

    xT_d = nc.dram_tensor("xT", [C, n], f32, kind="ExternalInput")
    maskT_d = nc.dram_tensor("maskT", [n, n], f32, kind="ExternalInput")
    wq_d = nc.dram_tensor("wq", [C + 1, HD], f32, kind="ExternalInput")
    wk_d = nc.dram_tensor("wk", [C + 1, HD], f32, kind="ExternalInput")
    wv_d = nc.dram_tensor("wv", [C + 1, VW], f32, kind="ExternalInput")
    wo_d = nc.dram_tensor("wo", [HD, C], f32, kind="ExternalInput")
    y_d = nc.dram_tensor("y", [n, C], f32, kind="ExternalOutput")

    with tile.TileContext(nc) as tc, ExitStack() as ctx:
        const = ctx.enter_context(tc.tile_pool(name="const", bufs=1))
        maskp = ctx.enter_context(tc.tile_pool(name="maskp", bufs=4))
        tpool = ctx.enter_context(tc.tile_pool(name="tpool", bufs=2))
        ppool = ctx.enter_context(tc.tile_pool(name="ppool", bufs=2))
        ypool = ctx.enter_context(tc.tile_pool(name="ypool", bufs=2))
        spool = ctx.enter_context(tc.tile_pool(name="spool", bufs=2))
        rpool = ctx.enter_context(tc.tile_pool(name="rpool", bufs=2))
        psT = ctx.enter_context(tc.tile_pool(name="psT", bufs=2, space="PSUM"))
        psO = ctx.enter_context(tc.tile_pool(name="psO", bufs=4, space="PSUM"))

        # ---------------- constants / inputs ----------------
        xc0 = const.tile([128, n], f32, tag="xc0")
        xc1 = const.tile([128, n], f32, tag="xc1")
        nc.sync.dma_start(xc0[:], xT_d[0:128, :])
        nc.sync.dma_start(xc1[:], xT_d[128:256, :])
        ones_row = const.tile([1, n], f32, tag="ones_row")
        nc.vector.memset(ones_row[:], 1.0)
        zrow = const.tile([1, CH], bf16, tag="zrow")
        nc.vector.memset(zrow[:], 0.0)
        zc = const.tile([1, 128], bf16, tag="zc")
        nc.vector.memset(zc[:], 0.0)

        wq_sb = const.tile([128, 2 * HD], f32, tag="wq_sb")
        wk_sb = const.tile([128, 2 * HD], f32, tag="wk_sb")
        wv_sb = const.tile([128, 2 * VW], f32, tag="wv_sb")
        wqb = const.tile([1, HD], f32, tag="wqb")
        wkb = const.tile([1, HD], f32, tag="wkb")
        wvb = const.tile([1, VW], f32, tag="wvb")
        wo_f = const.tile([128, C], f32, tag="wo_f")
        wo_sb = const.tile([128, C], f32r, tag="wo_sb")
        for sb, d_, w in ((wq_sb, wq_d, HD), (wk_sb, wk_d, HD), (wv_sb, wv_d, VW)):
            nc.sync.dma_start(sb[:, 0:w], d_[0:128, :])
            nc.sync.dma_start(sb[:, w:2 * w], d_[128:256, :])
        nc.sync.dma_start(wqb[:], wq_d[256:257, :])
        nc.sync.dma_start(wkb[:], wk_d[256:257, :])
        nc.sync.dma_start(wvb[:], wv_d[256:257, :])
        nc.sync.dma_start(wo_f[:], wo_d[:])
        nc.scalar.copy(wo_sb[:], wo_f[:])

        qT_sb = const.tile([128, n], f32r, tag="qT_sb")
        kT_sb = const.tile([128, n], f32r, tag="kT_sb")
        v_store = const.tile([128, MT * VW], f32r, tag="v_store")
        O_allT = const.tile([128, n], f32r, tag="O_allT")

        # f32r copies of the phase-1 matmul operands (DVE is idle here)
        xr0 = const.tile([128, n], f32r, tag="xr0")
        xr1 = const.tile([128, n], f32r, tag="xr1")
        ones_r = const.tile([1, n], f32r, tag="ones_r")
        wq_r = const.tile([128, 2 * HD], f32r, tag="wq_r")
        wk_r = const.tile([128, 2 * HD], f32r, tag="wk_r")
        wv_r = const.tile([128, 2 * VW], f32r, tag="wv_r")
        wqb_r = const.tile([1, HD], f32r, tag="wqb_r")
        wkb_r = const.tile([1, HD], f32r, tag="wkb_r")
        wvb_r = const.tile([1, VW], f32r, tag="wvb_r")
        nc.vector.tensor_copy(xr0[:], xc0[:])
        nc.vector.tensor_copy(xr1[:], xc1[:])
        nc.vector.tensor_copy(ones_r[:], ones_row[:])
        nc.vector.tensor_copy(wq_r[:], wq_sb[:])
        nc.vector.tensor_copy(wk_r[:], wk_sb[:])
        nc.vector.tensor_copy(wv_r[:], wv_sb[:])
        nc.vector.tensor_copy(wqb_r[:], wqb[:])
        nc.vector.tensor_copy(wkb_r[:], wkb[:])
        nc.vector.tensor_copy(wvb_r[:], wvb[:])

        for _rep in range(reps):
            # ------------- phase 1: QKV projections (plain fp32) -------------
            # qT/kT: [32h+d, n] = W.T @ x.T (+ bias via ones-row rank-1 term);
            # evictions round to f32r for the hot-loop matmuls.
            for chunk in range(NQ):
                cs = bass.ts(chunk, CH)
                for dst, w_sb, w_b in ((qT_sb, wq_r, wqb_r), (kT_sb, wk_r, wkb_r)):
                    pq = psO.tile([128, CH], f32, name="pq", tag="psO")
                    nc.tensor.matmul(pq[:], lhsT=w_sb[:, 0:HD], rhs=xr0[:, cs],
                                     start=True, stop=False)
                    nc.tensor.matmul(pq[:], lhsT=w_sb[:, HD:2 * HD],
                                     rhs=xr1[:, cs], start=False, stop=False)
                    nc.tensor.matmul(pq[:], lhsT=w_b[0:1, :],
                                     rhs=ones_r[0:1, cs],
                                     start=False, stop=True)
                    nc.scalar.copy(dst[:, cs], pq[:])
            # v: [m, (v_h | 1) x 4] per m-tile; ones col comes from the bias row
            for t in range(MT):
                ms = bass.ts(t, 128)
                pv = psO.tile([128, VW], f32, name="pv", tag="psO")
                nc.tensor.matmul(pv[:], lhsT=xr0[:, ms], rhs=wv_r[:, 0:VW],
                                 start=True, stop=False)
                nc.tensor.matmul(pv[:], lhsT=xr1[:, ms],
                                 rhs=wv_r[:, VW:2 * VW],
                                 start=False, stop=False)
                nc.tensor.matmul(pv[:], lhsT=ones_r[0:1, ms], rhs=wvb_r[0:1, :],
                                 start=False, stop=True)
                nc.vector.tensor_copy(v_store[:, t * VW:(t + 1) * VW], pv[:])

            # ------------- phase 2: attention, one pass per n-chunk ----------
            # The previous pass's epilogue is emitted after the first m-tile
            # pair of the next pass (engines execute in program order, so
            # this hides the serial sums->bcast->recip chain behind live
            # QK^T/mul work instead of stalling every engine at the
            # pass boundary).
            def epilogue_a(q, po):
                qs = bass.ts(q, CH)
                # sums (psum row 32 of each head) -> partition 32h (aligned)
                sraw = spool.tile([128, CH], f32, name="sraw", tag="sraw")
                for h in range(4):
                    nc.scalar.copy(sraw[32 * h:32 * h + 1, :],
                                   po[h][32:33, :])
                # broadcast each head's sums row across its 32 partitions (DMA)
                r_raw = rpool.tile([128, CH], f32, name="r_raw", tag="r_raw")
                src = sraw[:]
                bc = bass.AP(tensor=src.tensor, offset=src.offset,
                             ap=[[32 * src.ap[0][0], 4], [0, 32], src.ap[-1]])
                nc.sync.dma_start(r_raw[:], bc)
                r_scr = rpool.tile([128, CH], f32, name="r_scr", tag="r_scr")
                r_all = rpool.tile([128, CH], f32, name="r_all", tag="r_all")
                nc.vector.reciprocal_approx_accurate(r_all[:], r_raw[:],
                                                     r_scr[:])
                # evict + normalize O^T chunks (rounds to f32r for projection)
                for h in range(4):
                    nc.vector.tensor_mul(O_allT[32 * h:32 * h + 32, qs],
                                         po[h][0:32, :],
                                         r_all[32 * h:32 * h + 32, :])

            def epilogue_b(q):
                # output projection for this n-chunk
                py = psT.tile([128, 2 * CH], f32, name="py", tag="psT")
                for j in range(CH // 128):
                    ncol = q * CH + j * 128
                    nc.tensor.matmul(py[:, j * C:(j + 1) * C],
                                     lhsT=O_allT[:, ncol:ncol + 128],
                                     rhs=wo_sb[:], start=True, stop=True)
                y_sb = ypool.tile([128, (CH // 128) * C], f32, name="y_sb",
                                  tag="y_sb")
                nc.scalar.copy(y_sb[:], py[:, 0:(CH // 128) * C])
                for j in range(CH // 128):
                    nc.sync.dma_start(
                        y_d[q * CH + j * 128:q * CH + (j + 1) * 128, :],
                        y_sb[:, j * C:(j + 1) * C])

            pending = None
            for q in range(NQ):
                qs = bass.ts(q, CH)
                po = None
                pv_backlog = []
                for t in range(MT):
                    mask_t = maskp.tile([128, CH], f32, tag="mask_t")
                    nc.sync.dma_start(
                        mask_t[:],
                        maskT_d[t * 128:(t + 1) * 128, q * CH:(q + 1) * CH])
                    ti = t % TE
                    if ti == 0:
                        T_tile = tpool.tile([128, TE * 4 * CH], f32,
                                            name="T_tile", tag="T_tile")
                        P_tile = ppool.tile([128, TE * 4 * CH], f32r,
                                            name="P_tile", tag="P_tile")
                    for pair in range(2):
                        pt = psT.tile([128, 2 * CH], f32, name="pt", tag="psT")
                        for hh in range(2):
                            h = 2 * pair + hh
                            nc.tensor.matmul(
                                pt[:, hh * CH:(hh + 1) * CH],
                                lhsT=kT_sb[32 * h:32 * h + 32,
                                           t * 128:(t + 1) * 128],
                                rhs=qT_sb[32 * h:32 * h + 32, qs],
                                start=True, stop=True,
                                tile_position=(32 * h, 0))
                        # fused PSUM-evict + mask multiply (mask repeated 2x)
                        mrep = bass.AP(tensor=mask_t[:].tensor,
                                       offset=mask_t[:].offset,
                                       ap=[mask_t[:].ap[0], [0, 2],
                                           mask_t[:].ap[-1]])
                        dst = T_tile[:, (ti * 4 + pair * 2) * CH:
                                     (ti * 4 + pair * 2 + 2) * CH]
                        unit = (q * MT + t) * 2 + pair
                        if (unit * gp_frac) % 8 < gp_frac:
                            # relieve the DVE: ACT evicts PSUM, GPSIMD
                            # does the elementwise multiply in SBUF
                            nc.scalar.copy(dst, pt[:])
                            nc.gpsimd.tensor_mul(dst, dst, mrep)
                        else:
                            nc.vector.tensor_mul(dst, pt[:], mrep)
                    if ti == TE - 1:
                        nc.scalar.activation(P_tile[:], T_tile[:], Act.Exp)
                        pv_backlog.append((t - (TE - 1), P_tile))
                        if t == TE - 1:
                            # first m-tile pair of this pass is in flight:
                            # drain the previous pass's normalize chain, then
                            # set up this pass's PSUM accumulators.
                            if pending is not None:
                                epilogue_a(*pending)
                            po = [psO.tile([128, CH], f32, name="po",
                                           tag="psO") for _ in range(4)]
                            for h in range(4):
                                nc.tensor.matmul(po[h][:], lhsT=zc[0:1, :],
                                                 rhs=zrow[0:1, :],
                                                 start=True, stop=True)
                        if t == min(3 * TE - 1, MT - 1) and pending is not None:
                            # projection of the previous pass, late enough
                            # that its O_allT inputs are long since ready
                            epilogue_b(pending[0])
                            pending = None
                        for tb0, P_t in pv_backlog:
                            for tj in range(TE):
                                tb = tb0 + tj
                                for h in range(4):
                                    vs = v_store[:, tb * VW + 33 * h:
                                                 tb * VW + 33 * h + 33]
                                    nc.tensor.matmul(
                                        po[h][0:33, :],
                                        lhsT=vs,
                                        rhs=P_t[:, (tj * 4 + h) * CH:
                                                (tj * 4 + h + 1) * CH],
                                        start=False, stop=(tb == MT - 1),
                                        skip_group_check=True)
                        pv_backlog = []
                pending = (q, po)
            epilogue_a(*pending)
            epilogue_b(pending[0])
            pending = None
    nc.finalize()
    return nc


def host_prep(x, interaction_mask, W_qkv, b_qkv, W_out, b_out, n=N):
    """Build per-core input bindings (host-side sharding + layout prep)."""
    x = np.asarray(x, np.float32)
    interaction_mask = np.asarray(interaction_mask, np.float32)
    W_qkv = np.asarray(W_qkv, np.float32)
    b_qkv = np.asarray(b_qkv, np.float32)
    W_out = np.asarray(W_out, np.float32)

    maskT = np.ascontiguousarray(
        ((0.1 + 0.9 * interaction_mask) * SCALE).T).astype(np.float32)
    Wr = W_qkv.reshape(C, 3, H, D)
    br = b_qkv.reshape(3, H, D)
    Wor = W_out.reshape(H, D, C)

    in_maps = []
    for core in range(NCORES):
        b = core // 2
        g = core % 2
        hs = slice(4 * g, 4 * g + 4)
        xT = np.ascontiguousarray(x[b].T)  # [C, n]

        wq = np.concatenate([Wr[:, 0, hs, :].reshape(C, HD),
                             br[0, hs, :].reshape(1, HD)], axis=0)
        wk = np.concatenate([Wr[:, 1, hs, :].reshape(C, HD),
                             br[1, hs, :].reshape(1, HD)], axis=0)
        # v augmented with a ones column per head: weights 0, bias 1
        wv_blocks, bv_blocks = [], []
        for h in range(4 * g, 4 * g + 4):
            wv_blocks.append(np.concatenate(
                [Wr[:, 2, h, :], np.zeros((C, 1), np.float32)], axis=1))
            bv_blocks.append(np.concatenate(
                [br[2, h, :], np.ones((1,), np.float32)]))
        wv = np.concatenate(
            [np.concatenate(wv_blocks, axis=1),
             np.concatenate(bv_blocks)[None, :]], axis=0)  # [C+1, VW]
        wo = np.ascontiguousarray(Wor[hs].reshape(HD, C))

        in_maps.append({
            "xT": xT,
            "maskT": maskT,
            "wq": np.ascontiguousarray(wq),
            "wk": np.ascontiguousarray(wk),
            "wv": np.ascontiguousarray(wv),
            "wo": wo,
        })
    return in_maps


_PROGRAM = {}


def get_program(**kwargs):
    key = tuple(sorted(kwargs.items()))
    if key not in _PROGRAM:
        _PROGRAM[key] = build_program(**kwargs)
    return _PROGRAM[key]


def combine_outputs(results, b_out):
    """results: list of 8 per-core {name: np.ndarray}. Sums head-group
    partials per batch and adds the output bias."""
    b_out = np.asarray(b_out, np.float32)
    out = np.empty((B, N, C), np.float32)
    for b in range(B):
        out[b] = results[2 * b]["y"] + results[2 * b + 1]["y"] + b_out[None, :]
    return out


def kernel(x, interaction_mask, W_qkv, b_qkv, W_out, b_out):
    from concourse.bass_utils import run_bass_kernel_spmd

    in_maps = host_prep(x, interaction_mask, W_qkv, b_qkv, W_out, b_out)
    nc = get_program()
    res = run_bass_kernel_spmd(nc, in_maps, list(range(NCORES)))
    return combine_outputs(res.results, b_out)
